# revision 11
# baseline (speedup 1.0000x reference)
"""CrossTrajectoryAttention TRN2 kernel.

Sharding: 8 cores = 2 batches x 4 query-blocks of 392 tokens (2 frames).
Each core computes the full two-stage attention for its (batch, s-block):
  stage 1: per-frame spatial attention (queries = s-block, keys = all 1568
           tokens of the batch, softmax over the 196 tokens of each frame)
  diagonal gather + stage 2: temporal attention over the 8 frames per token.
The per-batch K/V projection is replicated across the 4 cores of a batch
(collective-free).

Layout strategy (all matmuls fp32r, full PE rate at N>=256):
  - stage-1 activations kept transposed ([dim, token]); the host supplies
    xq^T / xk^T so projection weights load untransposed as lhsT.
  - scores computed transposed ([keys, s]); exp on ACT reads 4 PSUM banks
    per call; row-sums (softmax denominators) come free from an appended
    ones-column in the PV matmul (M=65).
  - per-core key-frame order is permuted on the host so the diagonal
    (own-frame) slots are always 0 and 1 -> one program for all cores.
  - x (stage-1 output) goes to DRAM unnormalized; the 1/sumexp scaling is
    fused into stage-2's read-back as a DVE multiply with a DMA-broadcast
    reciprocal tile.
  - stage 2 computes q2/k2/v2 in natural [s, ...] layout (lhsT = x^T tiles);
    the tiny per-token frame attention runs on DVE (mul + segmented reduce).
"""

import numpy as np

B, S, DIM, H, FR = 2, 1568, 768, 12, 8
NTOK = 196  # spatial tokens per frame
D = 64  # head dim
SCALE = D ** -0.5
SBLK = 392  # queries per core
ST = 98  # s-tile
NT = 4  # s-tiles per core
JP = 6  # 128-row tiles of DIM
NCORES = 8

_CACHE = {}


def _apply_tile_patch():
    """This container's walrus rejects >1 sync-wait on the SP tail drain
    ("Too many sync wait commands").  Split the tail-drain waits so each
    drain instruction carries at most one."""
    import re
    import concourse.tile as tile
    from concourse.vector_clock import ScopedClock, VectorClock

    if getattr(tile.TileContext, "_drain_split_patched", False):
        return

    def _split_drain_and_barrier(self, tick_clock, wait_clock):
        gc = tick_clock.global_clock
        vals = [int(x) for x in re.findall(r"-?\d+", repr(gc))]
        for i, v in enumerate(vals):
            if v > 0:
                partial = [0] * len(vals)
                partial[i] = v
                d = self.nc.sync.drain()
                wait_clock.add_sem_waits(
                    d.ins, ScopedClock({None: VectorClock(partial)})
                )
        self.nc.sync.drain()
        self.nc.all_engine_barrier()
        assert self.sems is not None
        popped = self.nc._tile_sem_poison_stack.pop()
        assert popped is self._sem_poison
        self.nc.clear_and_free_semaphores(list(self.sems.allocated().values()))
        self.nc.all_engine_barrier()

    tile.TileContext._drain_and_barrier = _split_drain_and_barrier
    tile.TileContext._drain_split_patched = True


def _split_multiwait_bir(bir_bytes):
    """This walrus rejects instructions carrying more than one sync-wait
    ("Too many sync wait commands").  Hoist extra waits onto same-engine
    NoOp instructions inserted immediately before the original."""
    import json

    m = json.loads(bir_bytes)
    n = 0
    for fn in m["functions"]:
        for blk in fn["blocks"]:
            out = []
            for inst in blk["instructions"]:
                si = inst.get("sync_info")
                waits = (si or {}).get("on_wait") or []
                if len(waits) > 1:
                    for w in waits[:-1]:
                        n += 1
                        out.append({
                            "debug": inst.get("debug", 0),
                            "engine": inst["engine"],
                            "ins": [],
                            "outs": [],
                            "name": f"I-mw{n}",
                            "opcode": "NoOp",
                            "sync_info": {"on_update": [], "on_wait": [w]},
                        })
                    si["on_wait"] = [waits[-1]]
                out.append(inst)
            blk["instructions"] = out
    return json.dumps(m).encode()


def _apply_compile_patch():
    """Route bass2jax's BIR through _split_multiwait_bir before neuronxcc."""
    import concourse.bass2jax as bass2jax

    if getattr(bass2jax, "_multiwait_patched", False):
        return
    orig = bass2jax.compile_bir_kernel

    def patched(bir_json, tmpdir, neff_name="file.neff"):
        return orig(_split_multiwait_bir(bir_json), tmpdir,
                    neff_name=neff_name)

    bass2jax.compile_bir_kernel = patched
    bass2jax._multiwait_patched = True


def _build():
    """Build the (core-uniform) Bass program.  Returns the Bass object."""
    import concourse.bass as bass
    import concourse.mybir as mybir
    import concourse.tile as tile
    from concourse.masks import make_identity

    _apply_tile_patch()
    _apply_compile_patch()

    f32 = mybir.dt.float32
    f32r = mybir.dt.float32r
    Exp = mybir.ActivationFunctionType.Exp
    AX = mybir.AxisListType.X
    ADD = mybir.AluOpType.add

    nc = bass.Bass("TRN2", target_bir_lowering=False, debug=False,
                   num_devices=NCORES)

    xqT = nc.dram_tensor("xqT", [DIM, SBLK], f32, kind="ExternalInput").ap()
    xkT = nc.dram_tensor("xkT", [DIM, S], f32, kind="ExternalInput").ap()
    Wq = nc.dram_tensor("Wq", [DIM, DIM], f32, kind="ExternalInput").ap()
    Wkv = nc.dram_tensor("Wkv", [DIM, 2 * DIM], f32, kind="ExternalInput").ap()
    Wpq = nc.dram_tensor("Wpq", [DIM, DIM], f32, kind="ExternalInput").ap()
    Wpkv = nc.dram_tensor("Wpkv", [DIM, 2 * DIM], f32, kind="ExternalInput").ap()
    Wproj = nc.dram_tensor("Wproj", [DIM, DIM], f32, kind="ExternalInput").ap()
    bprojT = nc.dram_tensor("bprojT", [128, JP], f32, kind="ExternalInput").ap()

    outT = nc.dram_tensor("outT", [DIM, SBLK], f32, kind="ExternalOutput").ap()
    attn2o = nc.dram_tensor("attn2o", [SBLK, FR * H], f32,
                            kind="ExternalOutput").ap()

    def pbcast(row_ap, nparts):
        """[1, F] DRAM AP -> [nparts, F] partition-stride-0 AP."""
        return bass.AP(
            tensor=row_ap.tensor,
            offset=row_ap.offset,
            ap=[[0, nparts]] + [list(d) for d in row_ap.ap[1:]],
        )

    from contextlib import ExitStack
    with tile.TileContext(nc) as tc, ExitStack() as ctx:
        glob = ctx.enter_context(tc.tile_pool(name="glob", bufs=1))
        dram = ctx.enter_context(tc.tile_pool(name="dram", bufs=1, space="DRAM"))

        s1_sb = glob.tile([H * FR, SBLK], f32, tag="s1", name="s1")
        r_sb = glob.tile([H * FR, SBLK], f32, tag="rinv", name="rinv")
        ident = glob.tile([128, 128], f32, tag="ident", name="ident")
        bprj = glob.tile([128, JP], f32, tag="bprj", name="bprj")

        xu_d = dram.tile([H, 65, FR, SBLK], f32, tag="xu", name="xud")
        v2_d = dram.tile([SBLK, FR, DIM], f32, tag="v2", name="v2d")
        r_d = dram.tile([H * FR, SBLK], f32, tag="rd", name="rd")

        make_identity(nc, ident)
        nc.sync.dma_start(out=bprj, in_=bprojT)

        # pools alive for phases A+B, released before C
        ab = tc.tile_pool(name="ab", bufs=1)
        abp = ab.__enter__()
        qT_sb = abp.tile([128, JP, SBLK], f32r, tag="qT", name="qT")
        kT_sb = abp.tile([128, JP, 1664], f32r, tag="kT", name="kT")
        vA, vB = [], []
        for f in range(FR):
            va = abp.tile([128, H, 65], f32r, tag=f"vA{f}", name=f"vA{f}")
            vb = abp.tile([128, H, 65], f32r, tag=f"vB{f}", name=f"vB{f}")
            # rows 68..127 must be zero (padding keys); zero 64..128 first,
            # the later psum->sbuf copies rewrite rows 64..67.
            nc.vector.memset(vb[64:128, :, :].bitcast(f32), 0.0)
            nc.vector.memset(va[:, :, 64].bitcast(f32), 1.0)
            nc.vector.memset(vb[0:68, :, 64].bitcast(f32), 1.0)
            vA.append(va)
            vB.append(vb)

        # ================= phase A: stage-1 projections =================
        with tc.tile_pool(name="psA", bufs=3, space="PSUM") as psA:
            # qT = Wq^T @ xq^T
            with tc.tile_pool(name="loadq", bufs=1) as loadq:
                xqT_sb = loadq.tile([128, JP, SBLK], f32r, tag="xqT",
                                    name="xqT")
                Wq_sb = loadq.tile([128, JP, DIM], f32r, tag="Wq", name="Wq")
                for j in range(JP):
                    nc.sync.dma_start(
                        out=xqT_sb[:, j, :],
                        in_=xqT.bitcast(f32r).rearrange("(j p) s -> p j s", p=128)[:, j, :])
                    nc.sync.dma_start(
                        out=Wq_sb[:, j, :],
                        in_=Wq.bitcast(f32r).rearrange("(j p) m -> p j m", p=128)[:, j, :])
                for jm in range(JP):
                    ps = psA.tile([128, 512], f32, tag="psA", name="psA1")
                    ps = ps[:, :SBLK]
                    for jk in range(JP):
                        nc.tensor.matmul(
                            ps, lhsT=Wq_sb[:, jk, jm * 128:(jm + 1) * 128],
                            rhs=xqT_sb[:, jk, :],
                            start=(jk == 0), stop=(jk == JP - 1))
                    nc.any.tensor_copy(out=qT_sb[:, jm, :], in_=ps)

            # kT / v  (kT transposed w/ zero tail, v natural frame-aligned)
            with tc.tile_pool(name="loadk", bufs=1) as loadk:
                xkT_sb = loadk.tile([128, JP, S], f32r, tag="xkT", name="xkT")
                Wkv_sb = loadk.tile([128, JP, 2 * DIM], f32r, tag="Wkv",
                                    name="Wkv")
                for j in range(JP):
                    nc.sync.dma_start(
                        out=xkT_sb[:, j, :],
                        in_=xkT.bitcast(f32r).rearrange("(j p) s -> p j s", p=128)[:, j, :])
                    nc.sync.dma_start(
                        out=Wkv_sb[:, j, :],
                        in_=Wkv.bitcast(f32r).rearrange("(j p) m -> p j m", p=128)[:, j, :])
                nc.vector.memset(kT_sb[:, :, S:1664].bitcast(f32), 0.0)
                for jm in range(JP):
                    for mc in range(4):
                        m0, msz = mc * 512, min(512, S - mc * 512)
                        ps = psA.tile([128, 512], f32, tag="psA", name="psA2")
                        ps = ps[:, :msz]
                        for jk in range(JP):
                            nc.tensor.matmul(
                                ps,
                                lhsT=Wkv_sb[:, jk, jm * 128:(jm + 1) * 128],
                                rhs=xkT_sb[:, jk, m0:m0 + msz],
                                start=(jk == 0), stop=(jk == JP - 1))
                        nc.any.tensor_copy(out=kT_sb[:, jm, m0:m0 + msz],
                                           in_=ps)
                for f in range(FR):
                    for ch in range(2):
                        rows = 128 if ch == 0 else 68
                        t0 = f * NTOK + ch * 128
                        for dc in range(2):
                            d0, dsz = dc * 512, min(512, DIM - dc * 512)
                            ps = psA.tile([128, 512], f32, tag="psA",
                                          name="psA3")
                            ps = ps[:rows, :dsz]
                            for jk in range(JP):
                                nc.tensor.matmul(
                                    ps, lhsT=xkT_sb[:, jk, t0:t0 + rows],
                                    rhs=Wkv_sb[:, jk,
                                                 DIM + d0:DIM + d0 + dsz],
                                    start=(jk == 0), stop=(jk == JP - 1))
                            dst = (vA[f] if ch == 0 else vB[f])
                            nh = dsz // 64
                            nc.any.tensor_copy(
                                out=dst[:rows, d0 // 64:d0 // 64 + nh, 0:64],
                                in_=ps.rearrange("p (h d) -> p h d", d=64))

        # ================= phase B: stage-1 attention ===================
        with tc.tile_pool(name="smega", bufs=1, space="PSUM") as smega, \
             tc.tile_pool(name="pvp", bufs=2, space="PSUM") as pvp, \
             tc.tile_pool(name="expp", bufs=8) as expp, \
             tc.tile_pool(name="xup", bufs=3) as xup:
            for h in range(H):
                jh, po = h // 2, (h % 2) * 64
                qh = qT_sb[po:po + 64, jh, :]
                ets = []
                for g in range(4):  # frame pairs
                    sm = smega.tile([128, 4, 512], f32, tag="sm", name="sm")
                    for fi in range(2):
                        f = 2 * g + fi
                        for ch in range(2):
                            c0 = f * NTOK + ch * 128
                            nc.tensor.matmul(
                                sm[:, fi * 2 + ch, 0:SBLK],
                                lhsT=kT_sb[po:po + 64, jh, c0:c0 + 128],
                                rhs=qh, start=True, stop=True)
                    et = expp.tile([128, 4, SBLK], f32r, tag="et", name="et")
                    nc.scalar.activation(et, sm[:, :, 0:SBLK], Exp,
                                         scale=SCALE)
                    ets.append(et)
                for fp in range(4):
                    pv = pvp.tile([65, 2, 512], f32, tag="pv", name="pv")
                    for fi in range(2):
                        f = 2 * fp + fi
                        et = ets[f // 2]
                        for ch in range(2):
                            vv = vA[f] if ch == 0 else vB[f]
                            nc.tensor.matmul(
                                pv[0:65, fi, 0:SBLK],
                                lhsT=vv[:, h, :],
                                rhs=et[:, (f % 2) * 2 + ch, :],
                                start=(ch == 0), stop=(ch == 1))
                    xu = xup.tile([65, 2, SBLK], f32, tag="xu", name="xusb")
                    nc.vector.tensor_copy(out=xu, in_=pv[0:65, :, 0:SBLK])
                    nc.sync.dma_start(out=xu_d[h, :, 2 * fp:2 * fp + 2, :],
                                      in_=xu)
                nc.sync.dma_start(out=s1_sb[h * FR:(h + 1) * FR, :],
                                  in_=xu_d[h, 64, :, :])

        nc.vector.reciprocal(out=r_sb, in_=s1_sb)
        nc.sync.dma_start(out=r_d, in_=r_sb)

        ab.__exit__(None, None, None)  # release qT/kT/v before phase C

        # ================= phase C: stage 2 =============================
        persistC = ctx.enter_context(tc.tile_pool(name="persistC", bufs=1))
        s2outT = persistC.tile([128, JP, SBLK], f32r, tag="s2outT",
                               name="s2outT")
        q2_sb = [persistC.tile([ST, DIM], f32, tag=f"q2_{t}", name=f"q2_{t}")
                 for t in range(NT)]
        lg_sb = [persistC.tile([ST, FR, H], f32, tag=f"lg_{t}",
                               name=f"lg_{t}") for t in range(NT)]
        out2_sb = [persistC.tile([ST, DIM], f32, tag=f"o2_{t}",
                                 name=f"o2_{t}") for t in range(NT)]

        xtp = ctx.enter_context(tc.tile_pool(name="xtp", bufs=7))
        rbp = ctx.enter_context(tc.tile_pool(name="rbp", bufs=7))
        xnp = ctx.enter_context(tc.tile_pool(name="xnp", bufs=12))

        def load_xtn(f):
            """normalized x^T tiles [128, SBLK] for frame-slot f, per jp."""
            xtn = []
            for j in range(JP):
                raw = xtp.tile([128, SBLK], f32, tag="xt", name="xt")
                nc.sync.dma_start(out=raw[0:64, :],
                                  in_=xu_d[2 * j, 0:64, f, :])
                nc.sync.dma_start(out=raw[64:128, :],
                                  in_=xu_d[2 * j + 1, 0:64, f, :])
                rb = rbp.tile([128, SBLK], f32, tag="rb", name="rb")
                row0 = (2 * j) * FR + f
                row1 = (2 * j + 1) * FR + f
                nc.sync.dma_start(out=rb[0:64, :],
                                  in_=pbcast(r_d[row0:row0 + 1, :], 64))
                nc.sync.dma_start(out=rb[64:128, :],
                                  in_=pbcast(r_d[row1:row1 + 1, :], 64))
                xn = xnp.tile([128, SBLK], f32r, tag="xn", name="xn")
                nc.vector.tensor_mul(out=xn, in0=raw, in1=rb)
                xtn.append(xn)
            return xtn

        # pre-pass: q2 = scale * (x_diag @ Wpq); diag slots are 0 and 1
        with tc.tile_pool(name="wpq", bufs=1) as wpq, \
             tc.tile_pool(name="psCq", bufs=2, space="PSUM") as psCq:
            Wpq_sb = wpq.tile([128, JP, DIM], f32r, tag="Wpq", name="Wpq")
            for j in range(JP):
                nc.sync.dma_start(
                    out=Wpq_sb[:, j, :],
                    in_=Wpq.bitcast(f32r).rearrange("(j p) m -> p j m", p=128)[:, j, :])
            for slot in range(2):
                xtn = load_xtn(slot)
                for tt in range(2):
                    t = slot * 2 + tt
                    for dc in range(2):
                        d0, dsz = dc * 512, min(512, DIM - dc * 512)
                        ps = psCq.tile([128, 512], f32, tag="cq", name="cq")
                        ps = ps[0:ST, :dsz]
                        for jk in range(JP):
                            nc.tensor.matmul(
                                ps, lhsT=xtn[jk][:, t * ST:(t + 1) * ST],
                                rhs=Wpq_sb[:, jk, d0:d0 + dsz],
                                start=(jk == 0), stop=(jk == JP - 1))
                        nc.scalar.mul(q2_sb[t][:, d0:d0 + dsz], ps, SCALE)

        # main: k2 -> logits (streamed from PSUM), v2 -> DRAM
        with tc.tile_pool(name="wpkv", bufs=1) as wpkv, \
             tc.tile_pool(name="psCk", bufs=2, space="PSUM") as psCk, \
             tc.tile_pool(name="psCv", bufs=2, space="PSUM") as psCv, \
             tc.tile_pool(name="tmp1p", bufs=2) as tmp1p, \
             tc.tile_pool(name="stg", bufs=3) as stg:
            Wpkv_sb = wpkv.tile([128, JP, 2 * DIM], f32r, tag="Wpkv",
                                name="Wpkv")
            for j in range(JP):
                nc.sync.dma_start(
                    out=Wpkv_sb[:, j, :],
                    in_=Wpkv.bitcast(f32r).rearrange("(j p) m -> p j m", p=128)[:, j, :])
            for f in range(FR):
                xtn = load_xtn(f)
                for t in range(NT):
                    tsl = slice(t * ST, (t + 1) * ST)
                    tmp1 = tmp1p.tile([ST, DIM], f32, tag="tmp1", name="tmp1")
                    for dc in range(2):
                        d0, dsz = dc * 512, min(512, DIM - dc * 512)
                        ps = psCk.tile([128, 512], f32, tag="ck", name="ck")
                        ps = ps[0:ST, :dsz]
                        for jk in range(JP):
                            nc.tensor.matmul(
                                ps, lhsT=xtn[jk][:, tsl],
                                rhs=Wpkv_sb[:, jk, d0:d0 + dsz],
                                start=(jk == 0), stop=(jk == JP - 1))
                        nc.vector.tensor_mul(out=tmp1[:, d0:d0 + dsz],
                                             in0=ps,
                                             in1=q2_sb[t][:, d0:d0 + dsz])
                    nc.vector.tensor_reduce(
                        lg_sb[t][:, f, :],
                        tmp1.rearrange("p (h d) -> p h d", d=64),
                        axis=AX, op=ADD)
                    for dc in range(2):
                        d0, dsz = dc * 512, min(512, DIM - dc * 512)
                        ps = psCv.tile([128, 512], f32, tag="cv", name="cv")
                        ps = ps[0:ST, :dsz]
                        for jk in range(JP):
                            nc.tensor.matmul(
                                ps, lhsT=xtn[jk][:, tsl],
                                rhs=Wpkv_sb[:, jk,
                                              DIM + d0:DIM + d0 + dsz],
                                start=(jk == 0), stop=(jk == JP - 1))
                        st = stg.tile([ST, 512], f32, tag="stg", name="stg")
                        st = st[:, :dsz]
                        nc.any.tensor_copy(out=st, in_=ps)
                        nc.sync.dma_start(out=v2_d[tsl, f, d0:d0 + dsz],
                                          in_=st)

        # per-token frame softmax + weighted sum over frames
        with tc.tile_pool(name="smallp", bufs=4) as smallp, \
             tc.tile_pool(name="v2p", bufs=1) as v2p, \
             tc.tile_pool(name="tmp2p", bufs=1) as tmp2p:
            for t in range(NT):
                tsl = slice(t * ST, (t + 1) * ST)
                e2 = smallp.tile([ST, FR, H], f32, tag="e2", name="e2")
                nc.scalar.activation(e2, lg_sb[t], Exp)
                s2 = smallp.tile([ST, H], f32, tag="s2", name="s2")
                nc.vector.tensor_reduce(s2, e2.rearrange("p f h -> p h f"),
                                        axis=AX, op=ADD)
                r2 = smallp.tile([ST, H], f32, tag="r2", name="r2")
                nc.vector.reciprocal(out=r2, in_=s2)
                a2 = smallp.tile([ST, FR, H], f32, tag="a2", name="a2")
                nc.vector.tensor_mul(
                    out=a2, in0=e2,
                    in1=r2[:, None, :].to_broadcast((ST, FR, H)))
                nc.sync.dma_start(out=attn2o[tsl, :],
                                  in_=a2.rearrange("p f h -> p (f h)"))
                v2t = v2p.tile([ST, FR, DIM], f32, tag="v2t", name="v2t")
                nc.sync.dma_start(out=v2t, in_=v2_d[tsl, :, :])
                tmp2 = tmp2p.tile([ST, FR, DIM], f32, tag="tmp2", name="tmp2")
                nc.vector.tensor_mul(
                    out=tmp2.rearrange("p f (h d) -> p f h d", d=64),
                    in0=v2t.rearrange("p f (h d) -> p f h d", d=64),
                    in1=a2[:, :, :, None].to_broadcast((ST, FR, H, 64)))
                nc.vector.tensor_reduce(out2_sb[t],
                                        tmp2.rearrange("p f e -> p e f"),
                                        axis=AX, op=ADD)

        # transpose out2 -> s2outT, final projection + bias
        with tc.tile_pool(name="wproj", bufs=1) as wproj, \
             tc.tile_pool(name="psCf", bufs=2, space="PSUM") as psCf, \
             tc.tile_pool(name="otp", bufs=3) as otp:
            Wproj_sb = wproj.tile([128, JP, DIM], f32r, tag="Wproj",
                                  name="Wproj")
            for j in range(JP):
                nc.sync.dma_start(
                    out=Wproj_sb[:, j, :],
                    in_=Wproj.bitcast(f32r).rearrange("(j p) m -> p j m", p=128)[:, j, :])
            for t in range(NT):
                for j in range(JP):
                    ps = psCf.tile([128, 512], f32, tag="tr", name="tr")
                    ps = ps[:, 0:ST]
                    nc.tensor.transpose(ps,
                                        out2_sb[t][:, j * 128:(j + 1) * 128],
                                        ident[0:ST, 0:ST])
                    nc.any.tensor_copy(out=s2outT[:, j, t * ST:(t + 1) * ST],
                                       in_=ps)
            for jm in range(JP):
                ps = psCf.tile([128, 512], f32, tag="fo", name="fo")
                ps = ps[:, :SBLK]
                for jk in range(JP):
                    nc.tensor.matmul(
                        ps, lhsT=Wproj_sb[:, jk, jm * 128:(jm + 1) * 128],
                        rhs=s2outT[:, jk, :],
                        start=(jk == 0), stop=(jk == JP - 1))
                ot = otp.tile([128, SBLK], f32, tag="ot", name="ot")
                nc.vector.tensor_scalar_add(ot, ps,
                                            scalar1=bprj[:, jm:jm + 1])
                nc.sync.dma_start(
                    out=outT.rearrange("(j p) s -> p j s", p=128)[:, jm, :],
                    in_=ot)

    return nc


def _host_prep(xq, xk, Wq, Wkv, Wpq, Wpkv, Wproj, bproj):
    """Per-core input maps."""
    xq = np.asarray(xq, dtype=np.float32)
    xk = np.asarray(xk, dtype=np.float32)
    Wq = np.ascontiguousarray(np.asarray(Wq, dtype=np.float32))
    Wkv = np.ascontiguousarray(np.asarray(Wkv, dtype=np.float32))
    Wpq = np.ascontiguousarray(np.asarray(Wpq, dtype=np.float32))
    Wpkv = np.ascontiguousarray(np.asarray(Wpkv, dtype=np.float32))
    Wproj = np.ascontiguousarray(np.asarray(Wproj, dtype=np.float32))
    bprojT = np.ascontiguousarray(
        np.asarray(bproj, dtype=np.float32).reshape(JP, 128).T)

    in_maps, perms = [], []
    for c in range(NCORES):
        b, q = c // 4, c % 4
        perm = [2 * q, 2 * q + 1] + [f for f in range(FR)
                                     if f not in (2 * q, 2 * q + 1)]
        perms.append(perm)
        xqT_ = np.ascontiguousarray(xq[b, q * SBLK:(q + 1) * SBLK, :].T)
        xkp = xk[b].reshape(FR, NTOK, DIM)[perm].reshape(S, DIM)
        xkT_ = np.ascontiguousarray(xkp.T)
        in_maps.append({
            "xqT": xqT_, "xkT": xkT_, "Wq": Wq, "Wkv": Wkv, "Wpq": Wpq,
            "Wpkv": Wpkv, "Wproj": Wproj, "bprojT": bprojT,
        })
    return in_maps, perms


def kernel(xq, xk, Wq, Wkv, Wpq, Wpkv, Wproj, bproj, num_frames,
           trace=False, tmpdir=None):
    assert int(num_frames) == FR
    from concourse.bass_utils import run_bass_kernel_spmd

    if "nc" not in _CACHE:
        _CACHE["nc"] = _build()
    nc = _CACHE["nc"]

    in_maps, perms = _host_prep(xq, xk, Wq, Wkv, Wpq, Wpkv, Wproj, bproj)
    kwargs = {}
    if trace:
        kwargs.update(trace=True, tmpdir=tmpdir)
    res = run_bass_kernel_spmd(nc, in_maps, list(range(NCORES)), **kwargs)
    _CACHE["last_result"] = res

    out = np.empty((B, S, DIM), np.float32)
    attn2 = np.empty((B, H, S, FR), np.float32)
    for c in range(NCORES):
        b, q = c // 4, c % 4
        s0 = q * SBLK
        out[b, s0:s0 + SBLK, :] = res.results[c]["outT"].T
        a = res.results[c]["attn2o"].reshape(SBLK, FR, H).transpose(2, 0, 1)
        for slot in range(FR):
            attn2[b, :, s0:s0 + SBLK, perms[c][slot]] = a[:, :, slot]
    return out, attn2


# revision 15
# speedup vs baseline: 1.1933x; 1.1933x over previous
"""CrossTrajectoryAttention TRN2 kernel.

Sharding: 8 cores = 2 batches x 4 query-blocks of 392 tokens (2 frames).
Each core computes the full two-stage attention for its (batch, s-block):
  stage 1: per-frame spatial attention (queries = s-block, keys = all 1568
           tokens of the batch, softmax over the 196 tokens of each frame)
  diagonal gather + stage 2: temporal attention over the 8 frames per token.
The per-batch K/V projection is replicated across the 4 cores of a batch
(collective-free).

Layout strategy (all matmuls fp32r, full PE rate at N>=256):
  - stage-1 activations kept transposed ([dim, token]); the host supplies
    xq^T / xk^T so projection weights load untransposed as lhsT.
  - scores computed transposed ([keys, s]); exp on ACT reads 4 PSUM banks
    per call; row-sums (softmax denominators) come free from an appended
    ones-column in the PV matmul (M=65).
  - per-core key-frame order is permuted on the host so the diagonal
    (own-frame) slots are always 0 and 1 -> one program for all cores.
  - x (stage-1 output) goes to DRAM unnormalized; the 1/sumexp scaling is
    fused into stage-2's read-back as a DVE multiply with a DMA-broadcast
    reciprocal tile.
  - stage 2 computes q2/k2/v2 in natural [s, ...] layout (lhsT = x^T tiles);
    the tiny per-token frame attention runs on DVE (mul + segmented reduce).
"""

import numpy as np

B, S, DIM, H, FR = 2, 1568, 768, 12, 8
NTOK = 196  # spatial tokens per frame
D = 64  # head dim
SCALE = D ** -0.5
SBLK = 392  # queries per core
ST = 98  # s-tile
NT = 4  # s-tiles per core
JP = 6  # 128-row tiles of DIM
NCORES = 8

_CACHE = {}


def _apply_tile_patch():
    """This container's walrus rejects >1 sync-wait on the SP tail drain
    ("Too many sync wait commands").  Split the tail-drain waits so each
    drain instruction carries at most one."""
    import re
    import concourse.tile as tile
    from concourse.vector_clock import ScopedClock, VectorClock

    if getattr(tile.TileContext, "_drain_split_patched", False):
        return

    def _split_drain_and_barrier(self, tick_clock, wait_clock):
        gc = tick_clock.global_clock
        vals = [int(x) for x in re.findall(r"-?\d+", repr(gc))]
        for i, v in enumerate(vals):
            if v > 0:
                partial = [0] * len(vals)
                partial[i] = v
                d = self.nc.sync.drain()
                wait_clock.add_sem_waits(
                    d.ins, ScopedClock({None: VectorClock(partial)})
                )
        self.nc.sync.drain()
        self.nc.all_engine_barrier()
        assert self.sems is not None
        popped = self.nc._tile_sem_poison_stack.pop()
        assert popped is self._sem_poison
        self.nc.clear_and_free_semaphores(list(self.sems.allocated().values()))
        self.nc.all_engine_barrier()

    tile.TileContext._drain_and_barrier = _split_drain_and_barrier
    tile.TileContext._drain_split_patched = True


def _split_multiwait_bir(bir_bytes):
    """This walrus rejects instructions carrying more than one sync-wait
    ("Too many sync wait commands").  Hoist extra waits onto same-engine
    NoOp instructions inserted immediately before the original."""
    import json

    m = json.loads(bir_bytes)
    n = 0
    for fn in m["functions"]:
        for blk in fn["blocks"]:
            out = []
            for inst in blk["instructions"]:
                si = inst.get("sync_info")
                waits = (si or {}).get("on_wait") or []
                if len(waits) > 1:
                    for w in waits[:-1]:
                        n += 1
                        out.append({
                            "debug": inst.get("debug", 0),
                            "engine": inst["engine"],
                            "ins": [],
                            "outs": [],
                            "name": f"I-mw{n}",
                            "opcode": "NoOp",
                            "sync_info": {"on_update": [], "on_wait": [w]},
                        })
                    si["on_wait"] = [waits[-1]]
                out.append(inst)
            blk["instructions"] = out
    return json.dumps(m).encode()


def _apply_compile_patch():
    """Route bass2jax's BIR through _split_multiwait_bir before neuronxcc."""
    import concourse.bass2jax as bass2jax

    if getattr(bass2jax, "_multiwait_patched", False):
        return
    orig = bass2jax.compile_bir_kernel

    def patched(bir_json, tmpdir, neff_name="file.neff"):
        return orig(_split_multiwait_bir(bir_json), tmpdir,
                    neff_name=neff_name)

    bass2jax.compile_bir_kernel = patched
    bass2jax._multiwait_patched = True


def _build():
    """Build the (core-uniform) Bass program.  Returns the Bass object."""
    import concourse.bass as bass
    import concourse.mybir as mybir
    import concourse.tile as tile
    from concourse.masks import make_identity

    _apply_tile_patch()
    _apply_compile_patch()

    f32 = mybir.dt.float32
    bf16 = mybir.dt.bfloat16
    Exp = mybir.ActivationFunctionType.Exp
    AX = mybir.AxisListType.X
    ADD = mybir.AluOpType.add

    nc = bass.Bass("TRN2", target_bir_lowering=False, debug=False,
                   num_devices=NCORES)

    xqT = nc.dram_tensor("xqT", [DIM, SBLK], bf16, kind="ExternalInput").ap()
    xkT = nc.dram_tensor("xkT", [DIM, S], bf16, kind="ExternalInput").ap()
    Wq = nc.dram_tensor("Wq", [DIM, DIM], bf16, kind="ExternalInput").ap()
    Wkv = nc.dram_tensor("Wkv", [DIM, 2 * DIM], bf16, kind="ExternalInput").ap()
    Wpq = nc.dram_tensor("Wpq", [DIM, DIM], bf16, kind="ExternalInput").ap()
    Wpkv = nc.dram_tensor("Wpkv", [DIM, 2 * DIM], bf16,
                          kind="ExternalInput").ap()
    Wproj = nc.dram_tensor("Wproj", [DIM, DIM], bf16,
                           kind="ExternalInput").ap()
    bprojT = nc.dram_tensor("bprojT", [128, JP], f32, kind="ExternalInput").ap()

    outT = nc.dram_tensor("outT", [DIM, SBLK], f32, kind="ExternalOutput").ap()
    attn2o = nc.dram_tensor("attn2o", [SBLK, FR * H], f32,
                            kind="ExternalOutput").ap()

    def rawap(base_ap, off_el, dims):
        """Build an AP over base_ap's tensor: dims = [[stride_el, n], ...]."""
        return bass.AP(tensor=base_ap.tensor, offset=base_ap.offset + off_el,
                       ap=[list(d) for d in dims])

    from contextlib import ExitStack
    with tile.TileContext(nc) as tc, ExitStack() as ctx:
        glob = ctx.enter_context(tc.tile_pool(name="glob", bufs=1))
        dram = ctx.enter_context(tc.tile_pool(name="dram", bufs=1,
                                              space="DRAM"))

        s1_sb = glob.tile([H * FR, SBLK], bf16, tag="s1", name="s1")
        r_sb = glob.tile([H * FR, SBLK], bf16, tag="rinv", name="rinv")
        ident = glob.tile([128, 128], f32, tag="ident", name="ident")
        bprj = glob.tile([128, JP], f32, tag="bprj", name="bprj")

        xu_d = dram.tile([H, 65, FR, SBLK], bf16, tag="xu", name="xud")
        v2_d = dram.tile([SBLK, FR, DIM], bf16, tag="v2", name="v2d")
        r_d = dram.tile([H * FR, SBLK], bf16, tag="rd", name="rd")

        make_identity(nc, ident)
        nc.sync.dma_start(out=bprj, in_=bprojT)

        # pools alive for phases A+B, released before C
        ab = tc.tile_pool(name="ab", bufs=1)
        abp = ab.__enter__()
        qT_sb = abp.tile([128, JP, SBLK], bf16, tag="qT", name="qT")
        kT_sb = abp.tile([128, JP, 1664], bf16, tag="kT", name="kT")
        vA, vB = [], []
        for f in range(FR):
            va = abp.tile([128, H, 65], bf16, tag=f"vA{f}", name=f"vA{f}")
            vb = abp.tile([128, H, 65], bf16, tag=f"vB{f}", name=f"vB{f}")
            # rows 68..127 must be zero (padding keys); zero 64..128 first,
            # the later psum->sbuf copies rewrite rows 64..67.
            nc.vector.memset(vb[64:128, :, :], 0.0)
            nc.vector.memset(va[:, :, 64], 1.0)
            nc.vector.memset(vb[0:68, :, 64], 1.0)
            vA.append(va)
            vB.append(vb)

        # ================= phase A: stage-1 projections =================
        with tc.tile_pool(name="psA", bufs=4, space="PSUM") as psA:
            # qT = Wq^T @ xq^T
            with tc.tile_pool(name="loadq", bufs=1) as loadq:
                xqT_sb = loadq.tile([128, JP, SBLK], bf16, tag="xqT",
                                    name="xqT")
                Wq_sb = loadq.tile([128, JP, DIM], bf16, tag="Wq", name="Wq")
                for j in range(JP):
                    nc.sync.dma_start(
                        out=xqT_sb[:, j, :],
                        in_=xqT.rearrange("(j p) s -> p j s", p=128)[:, j, :])
                    nc.sync.dma_start(
                        out=Wq_sb[:, j, :],
                        in_=Wq.rearrange("(j p) m -> p j m", p=128)[:, j, :])
                for jm in range(JP):
                    ps = psA.tile([128, 512], f32, tag="psA", name="psA1")
                    ps = ps[:, :SBLK]
                    for jk in range(JP):
                        nc.tensor.matmul(
                            ps, lhsT=Wq_sb[:, jk, jm * 128:(jm + 1) * 128],
                            rhs=xqT_sb[:, jk, :],
                            start=(jk == 0), stop=(jk == JP - 1))
                    nc.any.tensor_copy(out=qT_sb[:, jm, :], in_=ps)

            # kT / v  (kT transposed w/ zero tail, v natural frame-aligned)
            with tc.tile_pool(name="loadk", bufs=1) as loadk:
                xkT_sb = loadk.tile([128, JP, S], bf16, tag="xkT", name="xkT")
                Wkv_sb = loadk.tile([128, JP, 2 * DIM], bf16, tag="Wkv",
                                    name="Wkv")
                for j in range(JP):
                    nc.sync.dma_start(
                        out=xkT_sb[:, j, :],
                        in_=xkT.rearrange("(j p) s -> p j s", p=128)[:, j, :])
                    nc.sync.dma_start(
                        out=Wkv_sb[:, j, :],
                        in_=Wkv.rearrange("(j p) m -> p j m", p=128)[:, j, :])
                nc.vector.memset(kT_sb[:, :, S:1664], 0.0)
                for jm in range(JP):
                    for mc in range(4):
                        m0, msz = mc * 512, min(512, S - mc * 512)
                        ps = psA.tile([128, 512], f32, tag="psA", name="psA2")
                        ps = ps[:, :msz]
                        for jk in range(JP):
                            nc.tensor.matmul(
                                ps,
                                lhsT=Wkv_sb[:, jk, jm * 128:(jm + 1) * 128],
                                rhs=xkT_sb[:, jk, m0:m0 + msz],
                                start=(jk == 0), stop=(jk == JP - 1))
                        nc.any.tensor_copy(out=kT_sb[:, jm, m0:m0 + msz],
                                           in_=ps)
                for f in range(FR):
                    for ch in range(2):
                        rows = 128 if ch == 0 else 68
                        t0 = f * NTOK + ch * 128
                        for dc in range(2):
                            d0, dsz = dc * 512, min(512, DIM - dc * 512)
                            ps = psA.tile([128, 512], f32, tag="psA",
                                          name="psA3")
                            ps = ps[:rows, :dsz]
                            for jk in range(JP):
                                nc.tensor.matmul(
                                    ps, lhsT=xkT_sb[:, jk, t0:t0 + rows],
                                    rhs=Wkv_sb[:, jk,
                                               DIM + d0:DIM + d0 + dsz],
                                    start=(jk == 0), stop=(jk == JP - 1))
                            dst = (vA[f] if ch == 0 else vB[f])
                            nh = dsz // 64
                            nc.any.tensor_copy(
                                out=dst[:rows, d0 // 64:d0 // 64 + nh, 0:64],
                                in_=ps.rearrange("p (h d) -> p h d", d=64))

        # ================= phase B: stage-1 attention ===================
        # head pairs interleaved so the K=64 score matmuls pack into
        # different PE row groups (base partitions 0 / 64) and overlap.
        with tc.tile_pool(name="smega", bufs=1, space="PSUM") as smega, \
             tc.tile_pool(name="pvp", bufs=2, space="PSUM") as pvp, \
             tc.tile_pool(name="expp", bufs=20) as expp, \
             tc.tile_pool(name="xup", bufs=4) as xup:
            for hp in range(JP):
                ets = [[], []]
                for f in range(FR):
                    sms = []
                    for i in range(2):
                        sm = smega.tile([128, 2, 512], f32,
                                        tag=f"sm{i}", name=f"sm{i}")
                        sms.append(sm)
                    for ch in range(2):
                        c0 = f * NTOK + ch * 128
                        for i in range(2):
                            po = i * 64
                            nc.tensor.matmul(
                                sms[i][:, ch, 0:SBLK],
                                lhsT=kT_sb[po:po + 64, hp, c0:c0 + 128],
                                rhs=qT_sb[po:po + 64, hp, :],
                                start=True, stop=True)
                    for i in range(2):
                        et = expp.tile([128, 2, SBLK], bf16, tag="et",
                                       name="et")
                        nc.scalar.activation(et, sms[i][:, :, 0:SBLK], Exp,
                                             scale=SCALE)
                        ets[i].append(et)
                for i in range(2):
                    h = 2 * hp + i
                    for fp in range(4):
                        pv = pvp.tile([65, 2, 512], f32, tag="pv", name="pv")
                        for fi in range(2):
                            f = 2 * fp + fi
                            for ch in range(2):
                                vv = vA[f] if ch == 0 else vB[f]
                                nc.tensor.matmul(
                                    pv[0:65, fi, 0:SBLK],
                                    lhsT=vv[:, h, :],
                                    rhs=ets[i][f][:, ch, :],
                                    start=(ch == 0), stop=(ch == 1))
                        xu = xup.tile([65, 2, SBLK], bf16, tag="xu",
                                      name="xusb")
                        nc.vector.tensor_copy(out=xu, in_=pv[0:65, :, 0:SBLK])
                        nc.gpsimd.dma_start(
                            out=xu_d[h, :, 2 * fp:2 * fp + 2, :], in_=xu)
                    nc.sync.dma_start(out=s1_sb[h * FR:(h + 1) * FR, :],
                                      in_=xu_d[h, 64, :, :])

        with nc.allow_low_precision(
                reason="softmax denominators; bf16 matches operand precision"):
            nc.vector.reciprocal(out=r_sb, in_=s1_sb)
        nc.gpsimd.dma_start(out=r_d, in_=r_sb)

        ab.__exit__(None, None, None)  # release qT/kT/v before phase C

        # ================= phase C: stage 2 =============================
        persistC = ctx.enter_context(tc.tile_pool(name="persistC", bufs=1))
        s2outT = persistC.tile([128, JP, SBLK], bf16, tag="s2outT",
                               name="s2outT")
        q2_sb = [persistC.tile([ST, DIM], f32, tag=f"q2_{t}", name=f"q2_{t}")
                 for t in range(NT)]
        lg_sb = [persistC.tile([ST, FR, H], f32, tag=f"lg_{t}",
                               name=f"lg_{t}") for t in range(NT)]
        out2_sb = [persistC.tile([ST, DIM], f32, tag=f"o2_{t}",
                                 name=f"o2_{t}") for t in range(NT)]

        xtp = ctx.enter_context(tc.tile_pool(name="xtp", bufs=3))
        rbp = ctx.enter_context(tc.tile_pool(name="rbp", bufs=3))
        xnp = ctx.enter_context(tc.tile_pool(name="xnp", bufs=3))

        HS = 65 * FR * SBLK  # head stride in xu_d elements

        def load_xtn(f):
            """Normalized x^T [128, JP, SBLK] for frame-slot f (one DMA for
            the raw x, one for the broadcast reciprocal, one DVE mul)."""
            raw = xtp.tile([128, JP, SBLK], bf16, tag="xt", name="xt")
            rb = rbp.tile([128, JP, SBLK], bf16, tag="rb", name="rb")
            for half in range(2):
                nc.sync.dma_start(
                    out=raw[half * 64:(half + 1) * 64, :, :],
                    in_=rawap(xu_d, half * HS + f * SBLK,
                              [[FR * SBLK, 64], [2 * HS, JP], [1, SBLK]]))
                nc.sync.dma_start(
                    out=rb[half * 64:(half + 1) * 64, :, :],
                    in_=rawap(r_d, (half * FR + f) * SBLK,
                              [[0, 64], [2 * FR * SBLK, JP], [1, SBLK]]))
            xn = xnp.tile([128, JP, SBLK], bf16, tag="xn", name="xn")
            nc.vector.tensor_mul(out=xn, in0=raw, in1=rb)
            return xn

        # pre-pass: q2 = scale * (x_diag @ Wpq); diag slots are 0 and 1
        with tc.tile_pool(name="wpq", bufs=1) as wpq, \
             tc.tile_pool(name="psCq", bufs=2, space="PSUM") as psCq:
            Wpq_sb = wpq.tile([128, JP, DIM], bf16, tag="Wpq", name="Wpq")
            for j in range(JP):
                nc.sync.dma_start(
                    out=Wpq_sb[:, j, :],
                    in_=Wpq.rearrange("(j p) m -> p j m", p=128)[:, j, :])
            for slot in range(2):
                xn = load_xtn(slot)
                for tt in range(2):
                    t = slot * 2 + tt
                    for dc in range(2):
                        d0, dsz = dc * 512, min(512, DIM - dc * 512)
                        ps = psCq.tile([128, 512], f32, tag="cq", name="cq")
                        ps = ps[0:ST, :dsz]
                        for jk in range(JP):
                            nc.tensor.matmul(
                                ps, lhsT=xn[:, jk, t * ST:(t + 1) * ST],
                                rhs=Wpq_sb[:, jk, d0:d0 + dsz],
                                start=(jk == 0), stop=(jk == JP - 1))
                        nc.scalar.mul(q2_sb[t][:, d0:d0 + dsz], ps, SCALE)

        # main: k2 -> logits (streamed from PSUM), v2 -> DRAM (bf16)
        with tc.tile_pool(name="wpkv", bufs=1) as wpkv, \
             tc.tile_pool(name="psCk", bufs=3, space="PSUM") as psCk, \
             tc.tile_pool(name="psCv", bufs=3, space="PSUM") as psCv, \
             tc.tile_pool(name="tmp1p", bufs=2) as tmp1p, \
             tc.tile_pool(name="stg", bufs=3) as stg:
            Wpkv_sb = wpkv.tile([128, JP, 2 * DIM], bf16, tag="Wpkv",
                                name="Wpkv")
            for j in range(JP):
                nc.sync.dma_start(
                    out=Wpkv_sb[:, j, :],
                    in_=Wpkv.rearrange("(j p) m -> p j m", p=128)[:, j, :])
            for f in range(FR):
                xn = load_xtn(f)
                for t in range(NT):
                    tsl = slice(t * ST, (t + 1) * ST)
                    tmp1 = tmp1p.tile([ST, DIM], f32, tag="tmp1", name="tmp1")
                    for dc in range(2):
                        d0, dsz = dc * 512, min(512, DIM - dc * 512)
                        ps = psCk.tile([128, 512], f32, tag="ck", name="ck")
                        ps = ps[0:ST, :dsz]
                        for jk in range(JP):
                            nc.tensor.matmul(
                                ps, lhsT=xn[:, jk, tsl],
                                rhs=Wpkv_sb[:, jk, d0:d0 + dsz],
                                start=(jk == 0), stop=(jk == JP - 1))
                        nc.vector.tensor_mul(out=tmp1[:, d0:d0 + dsz],
                                             in0=ps,
                                             in1=q2_sb[t][:, d0:d0 + dsz])
                    nc.vector.tensor_reduce(
                        lg_sb[t][:, f, :],
                        tmp1.rearrange("p (h d) -> p h d", d=64),
                        axis=AX, op=ADD)
                    st = stg.tile([ST, DIM], bf16, tag="stg", name="stg")
                    for dc in range(2):
                        d0, dsz = dc * 512, min(512, DIM - dc * 512)
                        ps = psCv.tile([128, 512], f32, tag="cv", name="cv")
                        ps = ps[0:ST, :dsz]
                        for jk in range(JP):
                            nc.tensor.matmul(
                                ps, lhsT=xn[:, jk, tsl],
                                rhs=Wpkv_sb[:, jk,
                                            DIM + d0:DIM + d0 + dsz],
                                start=(jk == 0), stop=(jk == JP - 1))
                        nc.any.tensor_copy(out=st[:, d0:d0 + dsz], in_=ps)
                    nc.gpsimd.dma_start(out=v2_d[tsl, f, :], in_=st)

        # per-token frame softmax + weighted sum over frames
        with tc.tile_pool(name="smallp", bufs=4) as smallp, \
             tc.tile_pool(name="v2p", bufs=2) as v2p, \
             tc.tile_pool(name="tmp2p", bufs=1) as tmp2p:
            for t in range(NT):
                tsl = slice(t * ST, (t + 1) * ST)
                e2 = smallp.tile([ST, FR, H], f32, tag="e2", name="e2")
                nc.scalar.activation(e2, lg_sb[t], Exp)
                s2 = smallp.tile([ST, H], f32, tag="s2", name="s2")
                nc.vector.tensor_reduce(s2, e2.rearrange("p f h -> p h f"),
                                        axis=AX, op=ADD)
                r2 = smallp.tile([ST, H], f32, tag="r2", name="r2")
                nc.vector.reciprocal(out=r2, in_=s2)
                a2 = smallp.tile([ST, FR, H], f32, tag="a2", name="a2")
                nc.vector.tensor_mul(
                    out=a2, in0=e2,
                    in1=r2[:, None, :].to_broadcast((ST, FR, H)))
                nc.gpsimd.dma_start(out=attn2o[tsl, :],
                                    in_=a2.rearrange("p f h -> p (f h)"))
                v2t = v2p.tile([ST, FR, DIM], bf16, tag="v2t", name="v2t")
                nc.sync.dma_start(out=v2t, in_=v2_d[tsl, :, :])
                tmp2 = tmp2p.tile([ST, FR, DIM], f32, tag="tmp2", name="tmp2")
                nc.vector.tensor_mul(
                    out=tmp2.rearrange("p f (h d) -> p f h d", d=64),
                    in0=v2t.rearrange("p f (h d) -> p f h d", d=64),
                    in1=a2[:, :, :, None].to_broadcast((ST, FR, H, 64)))
                nc.vector.tensor_reduce(out2_sb[t],
                                        tmp2.rearrange("p f e -> p e f"),
                                        axis=AX, op=ADD)

        # transpose out2 -> s2outT (bf16), final projection + bias
        with tc.tile_pool(name="wproj", bufs=1) as wproj, \
             tc.tile_pool(name="psCf", bufs=2, space="PSUM") as psCf, \
             tc.tile_pool(name="otp", bufs=3) as otp:
            Wproj_sb = wproj.tile([128, JP, DIM], bf16, tag="Wproj",
                                  name="Wproj")
            for j in range(JP):
                nc.sync.dma_start(
                    out=Wproj_sb[:, j, :],
                    in_=Wproj.rearrange("(j p) m -> p j m", p=128)[:, j, :])
            for t in range(NT):
                for j in range(JP):
                    ps = psCf.tile([128, 512], f32, tag="tr", name="tr")
                    ps = ps[:, 0:ST]
                    nc.tensor.transpose(ps,
                                        out2_sb[t][:, j * 128:(j + 1) * 128],
                                        ident[0:ST, 0:ST])
                    nc.any.tensor_copy(out=s2outT[:, j, t * ST:(t + 1) * ST],
                                       in_=ps)
            for jm in range(JP):
                ps = psCf.tile([128, 512], f32, tag="fo", name="fo")
                ps = ps[:, :SBLK]
                for jk in range(JP):
                    nc.tensor.matmul(
                        ps, lhsT=Wproj_sb[:, jk, jm * 128:(jm + 1) * 128],
                        rhs=s2outT[:, jk, :],
                        start=(jk == 0), stop=(jk == JP - 1))
                ot = otp.tile([128, SBLK], f32, tag="ot", name="ot")
                nc.vector.tensor_scalar_add(ot, ps,
                                            scalar1=bprj[:, jm:jm + 1])
                nc.gpsimd.dma_start(
                    out=outT.rearrange("(j p) s -> p j s", p=128)[:, jm, :],
                    in_=ot)

    return nc


def _host_prep(xq, xk, Wq, Wkv, Wpq, Wpkv, Wproj, bproj):
    """Per-core input maps (matmul operands pre-cast to bf16 on the host)."""
    import ml_dtypes

    bf = ml_dtypes.bfloat16
    xq = np.asarray(xq, dtype=np.float32)
    xk = np.asarray(xk, dtype=np.float32)
    Wq_ = np.ascontiguousarray(np.asarray(Wq, dtype=bf))
    Wkv_ = np.ascontiguousarray(np.asarray(Wkv, dtype=bf))
    Wpq_ = np.ascontiguousarray(np.asarray(Wpq, dtype=bf))
    Wpkv_ = np.ascontiguousarray(np.asarray(Wpkv, dtype=bf))
    Wproj_ = np.ascontiguousarray(np.asarray(Wproj, dtype=bf))
    bprojT = np.ascontiguousarray(
        np.asarray(bproj, dtype=np.float32).reshape(JP, 128).T)

    in_maps, perms = [], []
    for c in range(NCORES):
        b, q = c // 4, c % 4
        perm = [2 * q, 2 * q + 1] + [f for f in range(FR)
                                     if f not in (2 * q, 2 * q + 1)]
        perms.append(perm)
        xqT_ = np.ascontiguousarray(
            xq[b, q * SBLK:(q + 1) * SBLK, :].T.astype(bf))
        xkp = xk[b].reshape(FR, NTOK, DIM)[perm].reshape(S, DIM)
        xkT_ = np.ascontiguousarray(xkp.T.astype(bf))
        in_maps.append({
            "xqT": xqT_, "xkT": xkT_, "Wq": Wq_, "Wkv": Wkv_, "Wpq": Wpq_,
            "Wpkv": Wpkv_, "Wproj": Wproj_, "bprojT": bprojT,
        })
    return in_maps, perms


def kernel(xq, xk, Wq, Wkv, Wpq, Wpkv, Wproj, bproj, num_frames,
           trace=False, tmpdir=None):
    assert int(num_frames) == FR
    from concourse.bass_utils import run_bass_kernel_spmd

    if "nc" not in _CACHE:
        _CACHE["nc"] = _build()
    nc = _CACHE["nc"]

    in_maps, perms = _host_prep(xq, xk, Wq, Wkv, Wpq, Wpkv, Wproj, bproj)
    kwargs = {}
    if trace:
        kwargs.update(trace=True, tmpdir=tmpdir)
    res = run_bass_kernel_spmd(nc, in_maps, list(range(NCORES)), **kwargs)
    _CACHE["last_result"] = res

    out = np.empty((B, S, DIM), np.float32)
    attn2 = np.empty((B, H, S, FR), np.float32)
    for c in range(NCORES):
        b, q = c // 4, c % 4
        s0 = q * SBLK
        out[b, s0:s0 + SBLK, :] = res.results[c]["outT"].T
        a = res.results[c]["attn2o"].reshape(SBLK, FR, H).transpose(2, 0, 1)
        for slot in range(FR):
            attn2[b, :, s0:s0 + SBLK, perms[c][slot]] = a[:, :, slot]
    return out, attn2


# revision 20
# speedup vs baseline: 1.4741x; 1.2353x over previous
"""CrossTrajectoryAttention TRN2 kernel.

Sharding: 8 cores = 2 batches x 4 query-blocks of 392 tokens (2 frames).
Each core computes the full two-stage attention for its (batch, s-block):
  stage 1: per-frame spatial attention (queries = s-block, keys = all 1568
           tokens of the batch, softmax over the 196 tokens of each frame)
  diagonal gather + stage 2: temporal attention over the 8 frames per token.
The per-batch K/V projection is replicated across the 4 cores of a batch
(collective-free).

Layout strategy (all matmuls fp32r, full PE rate at N>=256):
  - stage-1 activations kept transposed ([dim, token]); the host supplies
    xq^T / xk^T so projection weights load untransposed as lhsT.
  - scores computed transposed ([keys, s]); exp on ACT reads 4 PSUM banks
    per call; row-sums (softmax denominators) come free from an appended
    ones-column in the PV matmul (M=65).
  - per-core key-frame order is permuted on the host so the diagonal
    (own-frame) slots are always 0 and 1 -> one program for all cores.
  - x (stage-1 output) goes to DRAM unnormalized; the 1/sumexp scaling is
    fused into stage-2's read-back as a DVE multiply with a DMA-broadcast
    reciprocal tile.
  - stage 2 computes q2/k2/v2 in natural [s, ...] layout (lhsT = x^T tiles);
    the tiny per-token frame attention runs on DVE (mul + segmented reduce).
"""

import numpy as np

B, S, DIM, H, FR = 2, 1568, 768, 12, 8
NTOK = 196  # spatial tokens per frame
D = 64  # head dim
SCALE = D ** -0.5
SBLK = 392  # queries per core
ST = 98  # s-tile
NT = 4  # s-tiles per core
JP = 6  # 128-row tiles of DIM
NCORES = 8

_CACHE = {}


def _apply_tile_patch():
    """This container's walrus rejects >1 sync-wait on the SP tail drain
    ("Too many sync wait commands").  Split the tail-drain waits so each
    drain instruction carries at most one."""
    import re
    import concourse.tile as tile
    from concourse.vector_clock import ScopedClock, VectorClock

    if getattr(tile.TileContext, "_drain_split_patched", False):
        return

    def _split_drain_and_barrier(self, tick_clock, wait_clock):
        gc = tick_clock.global_clock
        vals = [int(x) for x in re.findall(r"-?\d+", repr(gc))]
        for i, v in enumerate(vals):
            if v > 0:
                partial = [0] * len(vals)
                partial[i] = v
                d = self.nc.sync.drain()
                wait_clock.add_sem_waits(
                    d.ins, ScopedClock({None: VectorClock(partial)})
                )
        self.nc.sync.drain()
        self.nc.all_engine_barrier()
        assert self.sems is not None
        popped = self.nc._tile_sem_poison_stack.pop()
        assert popped is self._sem_poison
        self.nc.clear_and_free_semaphores(list(self.sems.allocated().values()))
        self.nc.all_engine_barrier()

    tile.TileContext._drain_and_barrier = _split_drain_and_barrier
    tile.TileContext._drain_split_patched = True


def _split_multiwait_bir(bir_bytes):
    """This walrus rejects instructions carrying more than one sync-wait
    ("Too many sync wait commands").  Hoist extra waits onto same-engine
    NoOp instructions inserted immediately before the original."""
    import json

    m = json.loads(bir_bytes)
    n = 0
    for fn in m["functions"]:
        for blk in fn["blocks"]:
            out = []
            for inst in blk["instructions"]:
                si = inst.get("sync_info")
                waits = (si or {}).get("on_wait") or []
                if len(waits) > 1:
                    for w in waits[:-1]:
                        n += 1
                        out.append({
                            "debug": inst.get("debug", 0),
                            "engine": inst["engine"],
                            "ins": [],
                            "outs": [],
                            "name": f"I-mw{n}",
                            "opcode": "NoOp",
                            "sync_info": {"on_update": [], "on_wait": [w]},
                        })
                    si["on_wait"] = [waits[-1]]
                out.append(inst)
            blk["instructions"] = out
    return json.dumps(m).encode()


def _apply_compile_patch():
    """Route bass2jax's BIR through _split_multiwait_bir before neuronxcc."""
    import concourse.bass2jax as bass2jax

    if getattr(bass2jax, "_multiwait_patched", False):
        return
    orig = bass2jax.compile_bir_kernel

    def patched(bir_json, tmpdir, neff_name="file.neff"):
        return orig(_split_multiwait_bir(bir_json), tmpdir,
                    neff_name=neff_name)

    bass2jax.compile_bir_kernel = patched
    bass2jax._multiwait_patched = True




def _build():
    """Build the (core-uniform) Bass program.  Returns the Bass object."""
    import concourse.bass as bass
    import concourse.mybir as mybir
    import concourse.tile as tile
    from concourse.masks import make_identity

    _apply_tile_patch()
    _apply_compile_patch()

    f32 = mybir.dt.float32
    bf16 = mybir.dt.bfloat16
    Exp = mybir.ActivationFunctionType.Exp
    AX = mybir.AxisListType.X
    ADD = mybir.AluOpType.add

    nc = bass.Bass("TRN2", target_bir_lowering=False, debug=False,
                   num_devices=NCORES)

    xqT = nc.dram_tensor("xqT", [DIM, SBLK], bf16, kind="ExternalInput").ap()
    xkT = nc.dram_tensor("xkT", [DIM, S], bf16, kind="ExternalInput").ap()
    Wq = nc.dram_tensor("Wq", [DIM, DIM], bf16, kind="ExternalInput").ap()
    Wkv = nc.dram_tensor("Wkv", [DIM, 2 * DIM], bf16, kind="ExternalInput").ap()
    Wpq = nc.dram_tensor("Wpq", [DIM, DIM], bf16, kind="ExternalInput").ap()
    Wpkv = nc.dram_tensor("Wpkv", [DIM, 2 * DIM], bf16,
                          kind="ExternalInput").ap()
    Wproj = nc.dram_tensor("Wproj", [DIM, DIM], bf16,
                           kind="ExternalInput").ap()
    bprojT = nc.dram_tensor("bprojT", [128, JP], f32, kind="ExternalInput").ap()

    outT = nc.dram_tensor("outT", [DIM, SBLK], f32, kind="ExternalOutput").ap()
    attn2o = nc.dram_tensor("attn2o", [SBLK, FR * H], f32,
                            kind="ExternalOutput").ap()

    def rawap(base_ap, off_el, dims):
        """Build an AP over base_ap's tensor: dims = [[stride_el, n], ...]."""
        return bass.AP(tensor=base_ap.tensor, offset=base_ap.offset + off_el,
                       ap=[list(d) for d in dims])

    from contextlib import ExitStack
    with tile.TileContext(nc) as tc, ExitStack() as ctx:
        glob = ctx.enter_context(tc.tile_pool(name="glob", bufs=1))
        dram = ctx.enter_context(tc.tile_pool(name="dram", bufs=1,
                                              space="DRAM"))

        s1_sb = glob.tile([H * FR, SBLK], bf16, tag="s1", name="s1")
        r_sb = glob.tile([H * FR, SBLK], bf16, tag="rinv", name="rinv")
        ident = glob.tile([128, 128], f32, tag="ident", name="ident")
        bprj = glob.tile([128, JP], f32, tag="bprj", name="bprj")

        xu_d = dram.tile([H, 65, FR, SBLK], bf16, tag="xu", name="xud")
        r_d = dram.tile([H * FR, SBLK], bf16, tag="rd", name="rd")

        make_identity(nc, ident)
        nc.sync.dma_start(out=bprj, in_=bprojT)

        # pools alive for phases A+B, released before C
        ab = tc.tile_pool(name="ab", bufs=1)
        abp = ab.__enter__()
        qT_sb = abp.tile([128, JP, SBLK], bf16, tag="qT", name="qT")
        kT_sb = abp.tile([128, JP, 1664], bf16, tag="kT", name="kT")
        vA, vB = [], []
        for f in range(FR):
            va = abp.tile([128, H, 65], bf16, tag=f"vA{f}", name=f"vA{f}")
            vb = abp.tile([128, H, 65], bf16, tag=f"vB{f}", name=f"vB{f}")
            # rows 68..127 must be zero (padding keys); zero 64..128 first,
            # the later psum->sbuf copies rewrite rows 64..67.
            nc.vector.memset(vb[64:128, :, :], 0.0)
            nc.vector.memset(va[:, :, 64], 1.0)
            nc.vector.memset(vb[0:68, :, 64], 1.0)
            vA.append(va)
            vB.append(vb)

        # ================= phase A: stage-1 projections =================
        with tc.tile_pool(name="psA", bufs=4, space="PSUM") as psA:
            # qT = Wq^T @ xq^T
            with tc.tile_pool(name="loadq", bufs=1) as loadq:
                xqT_sb = loadq.tile([128, JP, SBLK], bf16, tag="xqT",
                                    name="xqT")
                Wq_sb = loadq.tile([128, JP, DIM], bf16, tag="Wq", name="Wq")
                for j in range(JP):
                    nc.sync.dma_start(
                        out=xqT_sb[:, j, :],
                        in_=xqT.rearrange("(j p) s -> p j s", p=128)[:, j, :])
                    nc.sync.dma_start(
                        out=Wq_sb[:, j, :],
                        in_=Wq.rearrange("(j p) m -> p j m", p=128)[:, j, :])
                for jm in range(JP):
                    ps = psA.tile([128, 512], f32, tag="psA", name="psA1")
                    ps = ps[:, :SBLK]
                    for jk in range(JP):
                        nc.tensor.matmul(
                            ps, lhsT=Wq_sb[:, jk, jm * 128:(jm + 1) * 128],
                            rhs=xqT_sb[:, jk, :],
                            start=(jk == 0), stop=(jk == JP - 1))
                    nc.any.tensor_copy(out=qT_sb[:, jm, :], in_=ps)

            # kT / v  (kT transposed w/ zero tail, v natural frame-aligned)
            with tc.tile_pool(name="loadk", bufs=1) as loadk:
                xkT_sb = loadk.tile([128, JP, S], bf16, tag="xkT", name="xkT")
                Wkv_sb = loadk.tile([128, JP, 2 * DIM], bf16, tag="Wkv",
                                    name="Wkv")
                for j in range(JP):
                    nc.sync.dma_start(
                        out=xkT_sb[:, j, :],
                        in_=xkT.rearrange("(j p) s -> p j s", p=128)[:, j, :])
                    nc.sync.dma_start(
                        out=Wkv_sb[:, j, :],
                        in_=Wkv.rearrange("(j p) m -> p j m", p=128)[:, j, :])
                nc.vector.memset(kT_sb[:, :, S:1664], 0.0)
                for jm in range(JP):
                    pss = []
                    for mc in range(4):
                        msz = min(512, S - mc * 512)
                        ps = psA.tile([128, 512], f32, tag="psA", name="psA2")
                        pss.append(ps[:, :msz])
                    for jk in range(JP):
                        for mc in range(4):
                            m0, msz = mc * 512, min(512, S - mc * 512)
                            nc.tensor.matmul(
                                pss[mc],
                                lhsT=Wkv_sb[:, jk, jm * 128:(jm + 1) * 128],
                                rhs=xkT_sb[:, jk, m0:m0 + msz],
                                start=(jk == 0), stop=(jk == JP - 1))
                    for mc in range(4):
                        m0, msz = mc * 512, min(512, S - mc * 512)
                        nc.any.tensor_copy(out=kT_sb[:, jm, m0:m0 + msz],
                                           in_=pss[mc])
                for f in range(FR):
                    for ch in range(2):
                        rows = 128 if ch == 0 else 68
                        t0 = f * NTOK + ch * 128
                        pss = []
                        for dc in range(2):
                            dsz = min(512, DIM - dc * 512)
                            ps = psA.tile([128, 512], f32, tag="psA",
                                          name="psA3")
                            pss.append(ps[:rows, :dsz])
                        for jk in range(JP):
                            for dc in range(2):
                                d0, dsz = dc * 512, min(512, DIM - dc * 512)
                                nc.tensor.matmul(
                                    pss[dc], lhsT=xkT_sb[:, jk, t0:t0 + rows],
                                    rhs=Wkv_sb[:, jk,
                                               DIM + d0:DIM + d0 + dsz],
                                    start=(jk == 0), stop=(jk == JP - 1))
                        dst = (vA[f] if ch == 0 else vB[f])
                        for dc in range(2):
                            d0, dsz = dc * 512, min(512, DIM - dc * 512)
                            nh = dsz // 64
                            nc.any.tensor_copy(
                                out=dst[:rows, d0 // 64:d0 // 64 + nh, 0:64],
                                in_=pss[dc].rearrange("p (h d) -> p h d",
                                                      d=64))

        # ================= phase B: stage-1 attention ===================
        # head pairs interleaved so the K=64 score matmuls pack into
        # different PE row groups (base partitions 0 / 64) and overlap.
        with tc.tile_pool(name="smega", bufs=1, space="PSUM") as smega, \
             tc.tile_pool(name="pvp", bufs=2, space="PSUM") as pvp, \
             tc.tile_pool(name="expp", bufs=20) as expp, \
             tc.tile_pool(name="xup", bufs=4) as xup:
            for hp in range(JP):
                ets = [[], []]
                for f in range(FR):
                    sms = []
                    for i in range(2):
                        sm = smega.tile([128, 2, 512], f32,
                                        tag=f"sm{i}", name=f"sm{i}")
                        sms.append(sm)
                    for ch in range(2):
                        c0 = f * NTOK + ch * 128
                        for i in range(2):
                            po = i * 64
                            nc.tensor.matmul(
                                sms[i][:, ch, 0:SBLK],
                                lhsT=kT_sb[po:po + 64, hp, c0:c0 + 128],
                                rhs=qT_sb[po:po + 64, hp, :],
                                start=True, stop=True)
                    for i in range(2):
                        et = expp.tile([128, 2, SBLK], bf16, tag="et",
                                       name="et")
                        nc.scalar.activation(et, sms[i][:, :, 0:SBLK], Exp,
                                             scale=SCALE)
                        ets[i].append(et)
                for i in range(2):
                    h = 2 * hp + i
                    for fp in range(4):
                        pv = pvp.tile([65, 2, 512], f32, tag="pv", name="pv")
                        for fi in range(2):
                            f = 2 * fp + fi
                            for ch in range(2):
                                vv = vA[f] if ch == 0 else vB[f]
                                nc.tensor.matmul(
                                    pv[0:65, fi, 0:SBLK],
                                    lhsT=vv[:, h, :],
                                    rhs=ets[i][f][:, ch, :],
                                    start=(ch == 0), stop=(ch == 1))
                        xu = xup.tile([65, 2, SBLK], bf16, tag="xu",
                                      name="xusb")
                        if fp % 2 == 0:
                            nc.vector.tensor_copy(out=xu,
                                                  in_=pv[0:65, :, 0:SBLK])
                        else:
                            nc.scalar.copy(out=xu, in_=pv[0:65, :, 0:SBLK])
                        nc.gpsimd.dma_start(
                            out=xu_d[h, :, 2 * fp:2 * fp + 2, :], in_=xu)
                    nc.sync.dma_start(out=s1_sb[h * FR:(h + 1) * FR, :],
                                      in_=xu_d[h, 64, :, :])

        with nc.allow_low_precision(
                reason="softmax denominators; bf16 matches operand precision"):
            nc.vector.reciprocal(out=r_sb, in_=s1_sb)
        nc.gpsimd.dma_start(out=r_d, in_=r_sb)

        ab.__exit__(None, None, None)  # release qT/kT/v before phase C

        # ================= phase C: stage 2 =============================
        persistC = ctx.enter_context(tc.tile_pool(name="persistC", bufs=1))
        s2outT = persistC.tile([128, JP, SBLK], bf16, tag="s2outT",
                               name="s2outT")
        q2_sb = [persistC.tile([ST, DIM], f32, tag=f"q2_{t}", name=f"q2_{t}")
                 for t in range(NT)]
        lg_sb = [persistC.tile([ST, FR, H], f32, tag=f"lg_{t}",
                               name=f"lg_{t}") for t in range(NT)]
        out2_sb = [persistC.tile([ST, DIM], f32, tag=f"o2_{t}",
                                 name=f"o2_{t}") for t in range(NT)]

        xtp = ctx.enter_context(tc.tile_pool(name="xtp", bufs=3))
        rbp = ctx.enter_context(tc.tile_pool(name="rbp", bufs=3))
        xnp = ctx.enter_context(tc.tile_pool(name="xnp", bufs=10))

        HS = 65 * FR * SBLK  # head stride in xu_d elements

        def load_xtn(f):
            """Normalized x^T [128, JP, SBLK] for frame-slot f (one DMA for
            the raw x, one for the broadcast reciprocal, one DVE mul)."""
            raw = xtp.tile([128, JP, SBLK], bf16, tag="xt", name="xt")
            rb = rbp.tile([128, JP, SBLK], bf16, tag="rb", name="rb")
            for half in range(2):
                nc.sync.dma_start(
                    out=raw[half * 64:(half + 1) * 64, :, :],
                    in_=rawap(xu_d, half * HS + f * SBLK,
                              [[FR * SBLK, 64], [2 * HS, JP], [1, SBLK]]))
                nc.sync.dma_start(
                    out=rb[half * 64:(half + 1) * 64, :, :],
                    in_=rawap(r_d, (half * FR + f) * SBLK,
                              [[0, 64], [2 * FR * SBLK, JP], [1, SBLK]]))
            xn = xnp.tile([128, JP, SBLK], bf16, tag="xn", name="xn")
            nc.vector.tensor_mul(out=xn, in0=raw, in1=rb)
            return xn

        # pre-pass: q2 = scale * (x_diag @ Wpq); diag slots are 0 and 1
        with tc.tile_pool(name="wpq", bufs=1) as wpq, \
             tc.tile_pool(name="psCq", bufs=2, space="PSUM") as psCq:
            Wpq_sb = wpq.tile([128, JP, DIM], bf16, tag="Wpq", name="Wpq")
            for j in range(JP):
                nc.sync.dma_start(
                    out=Wpq_sb[:, j, :],
                    in_=Wpq.rearrange("(j p) m -> p j m", p=128)[:, j, :])
            xns = [load_xtn(f) for f in range(FR)]
            for slot in range(2):
                xn = xns[slot]
                for tt in range(2):
                    t = slot * 2 + tt
                    pss = []
                    for dc in range(2):
                        dsz = min(512, DIM - dc * 512)
                        ps = psCq.tile([128, 512], f32, tag="cq", name="cq")
                        pss.append(ps[0:ST, :dsz])
                    for jk in range(JP):
                        for dc in range(2):
                            d0, dsz = dc * 512, min(512, DIM - dc * 512)
                            nc.tensor.matmul(
                                pss[dc], lhsT=xn[:, jk, t * ST:(t + 1) * ST],
                                rhs=Wpq_sb[:, jk, d0:d0 + dsz],
                                start=(jk == 0), stop=(jk == JP - 1))
                    for dc in range(2):
                        d0, dsz = dc * 512, min(512, DIM - dc * 512)
                        nc.scalar.mul(q2_sb[t][:, d0:d0 + dsz], pss[dc],
                                      SCALE)

        # main: k2 -> logits (streamed from PSUM), v2 -> DRAM (bf16)
        with tc.tile_pool(name="wpkv", bufs=1) as wpkv, \
             tc.tile_pool(name="psCk", bufs=3, space="PSUM") as psCk, \
             tc.tile_pool(name="psCv", bufs=3, space="PSUM") as psCv, \
             tc.tile_pool(name="tmp1p", bufs=2) as tmp1p, \
             tc.tile_pool(name="smallp", bufs=4) as smallp, \
             tc.tile_pool(name="tmp2p", bufs=1) as tmp2p:
            Wpkv_sb = wpkv.tile([128, JP, 2 * DIM], bf16, tag="Wpkv",
                                name="Wpkv")
            for j in range(JP):
                nc.sync.dma_start(
                    out=Wpkv_sb[:, j, :],
                    in_=Wpkv.rearrange("(j p) m -> p j m", p=128)[:, j, :])
            for f in range(FR):
                for t in range(NT):
                    tsl = slice(t * ST, (t + 1) * ST)
                    tmp1 = tmp1p.tile([ST, DIM], f32, tag="tmp1", name="tmp1")
                    pss = []
                    for dc in range(2):
                        dsz = min(512, DIM - dc * 512)
                        ps = psCk.tile([128, 512], f32, tag="ck", name="ck")
                        pss.append(ps[0:ST, :dsz])
                    for jk in range(JP):
                        for dc in range(2):
                            d0, dsz = dc * 512, min(512, DIM - dc * 512)
                            nc.tensor.matmul(
                                pss[dc], lhsT=xns[f][:, jk, tsl],
                                rhs=Wpkv_sb[:, jk, d0:d0 + dsz],
                                start=(jk == 0), stop=(jk == JP - 1))
                    for dc in range(2):
                        d0, dsz = dc * 512, min(512, DIM - dc * 512)
                        nc.vector.tensor_mul(out=tmp1[:, d0:d0 + dsz],
                                             in0=pss[dc],
                                             in1=q2_sb[t][:, d0:d0 + dsz])
                    nc.vector.tensor_reduce(
                        lg_sb[t][:, f, :],
                        tmp1.rearrange("p (h d) -> p h d", d=64),
                        axis=AX, op=ADD)

            # frame softmax per s-tile; a2 kept for the fused v2 pass
            a2s = []
            for t in range(NT):
                tsl = slice(t * ST, (t + 1) * ST)
                e2 = smallp.tile([ST, FR, H], f32, tag="e2", name="e2")
                nc.scalar.activation(e2, lg_sb[t], Exp)
                s2 = smallp.tile([ST, H], f32, tag="s2", name="s2")
                nc.vector.tensor_reduce(s2, e2.rearrange("p f h -> p h f"),
                                        axis=AX, op=ADD)
                r2 = smallp.tile([ST, H], f32, tag="r2", name="r2")
                nc.vector.reciprocal(out=r2, in_=s2)
                a2 = smallp.tile([ST, FR, H], f32, tag=f"a2_{t}",
                                 name=f"a2_{t}")
                nc.vector.tensor_mul(
                    out=a2, in0=e2,
                    in1=r2[:, None, :].to_broadcast((ST, FR, H)))
                nc.gpsimd.dma_start(out=attn2o[tsl, :],
                                    in_=a2.rearrange("p f h -> p (f h)"))
                a2s.append(a2)

            # fused v2 projection + attn2 weighting + tree-sum over frames
            for t in range(NT):
                tsl = slice(t * ST, (t + 1) * ST)
                tmp2 = tmp2p.tile([ST, FR, DIM], f32, tag="tmp2", name="tmp2")
                for f in range(FR):
                    pss = []
                    for dc in range(2):
                        dsz = min(512, DIM - dc * 512)
                        ps = psCv.tile([128, 512], f32, tag="cv", name="cv")
                        pss.append(ps[0:ST, :dsz])
                    for jk in range(JP):
                        for dc in range(2):
                            d0, dsz = dc * 512, min(512, DIM - dc * 512)
                            nc.tensor.matmul(
                                pss[dc], lhsT=xns[f][:, jk, tsl],
                                rhs=Wpkv_sb[:, jk,
                                            DIM + d0:DIM + d0 + dsz],
                                start=(jk == 0), stop=(jk == JP - 1))
                    for dc in range(2):
                        d0, dsz = dc * 512, min(512, DIM - dc * 512)
                        nh = dsz // 64
                        h0 = d0 // 64
                        nc.vector.tensor_mul(
                            out=tmp2[:, f, d0:d0 + dsz].rearrange(
                                "p (h e) -> p h e", e=64),
                            in0=pss[dc].rearrange("p (h e) -> p h e", e=64),
                            in1=a2s[t][:, f, h0:h0 + nh, None].to_broadcast(
                                (ST, nh, 64)))
                su4 = tmp2p.tile([ST, 4, DIM], f32, tag="su4", name="su4")
                nc.vector.tensor_add(su4, tmp2[:, 0:4, :], tmp2[:, 4:8, :])
                su2 = tmp2p.tile([ST, 2, DIM], f32, tag="su2", name="su2")
                nc.vector.tensor_add(su2, su4[:, 0:2, :], su4[:, 2:4, :])
                nc.vector.tensor_add(out2_sb[t], su2[:, 0, :], su2[:, 1, :])

        # transpose out2 -> s2outT (bf16), final projection + bias
        with tc.tile_pool(name="wproj", bufs=1) as wproj, \
             tc.tile_pool(name="psCf", bufs=2, space="PSUM") as psCf, \
             tc.tile_pool(name="otp", bufs=3) as otp:
            Wproj_sb = wproj.tile([128, JP, DIM], bf16, tag="Wproj",
                                  name="Wproj")
            for j in range(JP):
                nc.sync.dma_start(
                    out=Wproj_sb[:, j, :],
                    in_=Wproj.rearrange("(j p) m -> p j m", p=128)[:, j, :])
            for t in range(NT):
                for j in range(JP):
                    ps = psCf.tile([128, 512], f32, tag="tr", name="tr")
                    ps = ps[:, 0:ST]
                    nc.tensor.transpose(ps,
                                        out2_sb[t][:, j * 128:(j + 1) * 128],
                                        ident[0:ST, 0:ST])
                    nc.any.tensor_copy(out=s2outT[:, j, t * ST:(t + 1) * ST],
                                       in_=ps)
            for jm in range(JP):
                ps = psCf.tile([128, 512], f32, tag="fo", name="fo")
                ps = ps[:, :SBLK]
                for jk in range(JP):
                    nc.tensor.matmul(
                        ps, lhsT=Wproj_sb[:, jk, jm * 128:(jm + 1) * 128],
                        rhs=s2outT[:, jk, :],
                        start=(jk == 0), stop=(jk == JP - 1))
                ot = otp.tile([128, SBLK], f32, tag="ot", name="ot")
                nc.vector.tensor_scalar_add(ot, ps,
                                            scalar1=bprj[:, jm:jm + 1])
                nc.gpsimd.dma_start(
                    out=outT.rearrange("(j p) s -> p j s", p=128)[:, jm, :],
                    in_=ot)

    return nc


def _host_prep(xq, xk, Wq, Wkv, Wpq, Wpkv, Wproj, bproj):
    """Per-core input maps (matmul operands pre-cast to bf16 on the host)."""
    import ml_dtypes

    bf = ml_dtypes.bfloat16
    xq = np.asarray(xq, dtype=np.float32)
    xk = np.asarray(xk, dtype=np.float32)
    Wq_ = np.ascontiguousarray(np.asarray(Wq, dtype=bf))
    Wkv_ = np.ascontiguousarray(np.asarray(Wkv, dtype=bf))
    Wpq_ = np.ascontiguousarray(np.asarray(Wpq, dtype=bf))
    Wpkv_ = np.ascontiguousarray(np.asarray(Wpkv, dtype=bf))
    Wproj_ = np.ascontiguousarray(np.asarray(Wproj, dtype=bf))
    bprojT = np.ascontiguousarray(
        np.asarray(bproj, dtype=np.float32).reshape(JP, 128).T)

    in_maps, perms = [], []
    for c in range(NCORES):
        b, q = c // 4, c % 4
        perm = [2 * q, 2 * q + 1] + [f for f in range(FR)
                                     if f not in (2 * q, 2 * q + 1)]
        perms.append(perm)
        xqT_ = np.ascontiguousarray(
            xq[b, q * SBLK:(q + 1) * SBLK, :].T.astype(bf))
        xkp = xk[b].reshape(FR, NTOK, DIM)[perm].reshape(S, DIM)
        xkT_ = np.ascontiguousarray(xkp.T.astype(bf))
        in_maps.append({
            "xqT": xqT_, "xkT": xkT_, "Wq": Wq_, "Wkv": Wkv_, "Wpq": Wpq_,
            "Wpkv": Wpkv_, "Wproj": Wproj_, "bprojT": bprojT,
        })
    return in_maps, perms


def kernel(xq, xk, Wq, Wkv, Wpq, Wpkv, Wproj, bproj, num_frames,
           trace=False, tmpdir=None):
    assert int(num_frames) == FR
    from concourse.bass_utils import run_bass_kernel_spmd

    if "nc" not in _CACHE:
        _CACHE["nc"] = _build()
    nc = _CACHE["nc"]

    in_maps, perms = _host_prep(xq, xk, Wq, Wkv, Wpq, Wpkv, Wproj, bproj)
    kwargs = {}
    if trace:
        kwargs.update(trace=True, tmpdir=tmpdir)
    res = run_bass_kernel_spmd(nc, in_maps, list(range(NCORES)), **kwargs)
    _CACHE["last_result"] = res

    out = np.empty((B, S, DIM), np.float32)
    attn2 = np.empty((B, H, S, FR), np.float32)
    for c in range(NCORES):
        b, q = c // 4, c % 4
        s0 = q * SBLK
        out[b, s0:s0 + SBLK, :] = res.results[c]["outT"].T
        a = res.results[c]["attn2o"].reshape(SBLK, FR, H).transpose(2, 0, 1)
        for slot in range(FR):
            attn2[b, :, s0:s0 + SBLK, perms[c][slot]] = a[:, :, slot]
    return out, attn2
